# revision 29
# baseline (speedup 1.0000x reference)
"""Sharded kNN (ArgDistanceMeasure) on 8 TRN2 NeuronCores.

Strategy (FAISS-style sharded kNN):
  - b (the database, [65536, 512]) is sharded row-wise across 8 cores
    (8192 rows each); a (queries, [2048, 512]) is replicated.
  - Ranking identity: argmin_j ||a_i - b_j + eps||^2 over j only needs the
    column-dependent part  score[i,j] = 2*a_i.b_j - (||b_j||^2 - 2*eps*sum(b_j)),
    maximized.  The row-constant terms (||a_i||^2 etc.) don't affect per-row
    ranking.
  - Per [128 queries x 2048 cols] chunk (four-engine pipeline, all ~235us):
      PE:  bf16 GEMM accumulating 2*cross into PSUM (4 K-tiles, N=512).
      ACT: copy PSUM -> SBUF, casting to fp16.
      DVE/GPS (alternating): subtract the replicated per-column bias (fp16).
      DVE: two pairwise-max levels (2048 -> 1024 -> 512, fp16 TT 2x mode),
           then max8 + find_index8 over the 512 quad-maxima.
  - Each winner expands to its 4 possible columns on the host, which
    recomputes the exact fp32 reference distance for the ~1024
    candidates/query, picks the final top-n with the reference's tie-break,
    and applies the reference's buggy index bookkeeping.  (bf16 GEMM noise +
    fp16 quantization + quad-expansion are provably safe on this data: zero
    true top-16 members lost in simulation.)
"""

import numpy as np

NA, D, NB = 2048, 512, 65536
NCORES = 8
NB_SHARD = NB // NCORES  # 8192
CHUNK = 2048             # chunk width (4 PSUM banks)
QUAD = CHUNK // 4        # 512 quad-maxima per chunk
TOP = 8                  # top-8 per chunk (vector.max width)
EPS = 1e-6


def build_kernel(na=NA, nb_shard=NB_SHARD, chunk=CHUNK):
    import concourse.mybir as mybir
    from concourse import bacc
    from concourse.tile import TileContext

    BF = mybir.dt.bfloat16
    F16 = mybir.dt.float16
    F32 = mybir.dt.float32
    U32 = mybir.dt.uint32

    nseg = nb_shard // chunk
    nsub = chunk // 512
    half = chunk // 2
    quad = chunk // 4
    kt = D // 128
    mt = na // 128

    # Bacc (not plain Bass): its compile() pipeline moves matmul waits onto
    # ldweights and splits multi-wait sync via event semaphores — TRN2
    # instructions encode at most ONE sync wait.
    nc = bacc.Bacc()

    # bT is packed chunk-column-major (all k-tiles of one 2048-column chunk
    # group contiguous) and split into one DRAM param + DMA per chunk group,
    # so the PE can start on chunk 0 long before the whole database loads.
    # Chunk group 0 is additionally split per k-tile for the earliest start.
    bt0_p = [
        nc.declare_dram_parameter(f"bt0k{k}", [128, chunk], BF, isOutput=False)
        for k in range(kt)
    ]
    bts_p = [
        nc.declare_dram_parameter(f"bt{g}", [128, kt * chunk], BF, isOutput=False)
        for g in range(1, nseg)
    ]
    at_p = nc.declare_dram_parameter("at", [128, kt * na], BF, isOutput=False)
    # Per-column bias replicated across partitions, fp16.
    crep_p = nc.declare_dram_parameter("crep", [128, nb_shard], F16, isOutput=False)
    out_val = nc.declare_dram_parameter("out_val", [na, nseg * TOP], F16, isOutput=True)
    out_idx = nc.declare_dram_parameter("out_idx", [na, nseg * TOP], U32, isOutput=True)

    with TileContext(nc) as tc:
        with (
            tc.tile_pool(name="weights", bufs=1) as wpool,
            tc.tile_pool(name="psum", bufs=2, space="PSUM") as ppool,
            tc.tile_pool(name="scores", bufs=6) as spool,
            tc.tile_pool(name="pairs", bufs=6) as mpool,
            tc.tile_pool(name="win", bufs=6) as winpool,
        ):
            at_sb = wpool.tile([128, kt * na], BF, tag="at")
            nc.sync.dma_start(out=at_sb, in_=at_p[:, :])
            bt0k = []
            for k in range(kt):
                t = wpool.tile([128, chunk], BF, tag=f"bt0k{k}")
                nc.sync.dma_start(out=t, in_=bt0_p[k][:, :])
                bt0k.append(t)
            crep = wpool.tile([128, nb_shard], F16, tag="crep")
            nc.sync.dma_start(out=crep, in_=crep_p[:, :])
            bts = [None]
            for g in range(1, nseg):
                t = wpool.tile([128, kt * chunk], BF, tag=f"bt{g}")
                nc.sync.dma_start(out=t, in_=bts_p[g - 1][:, :])
                bts.append(t)

            def bt_slice(s, k, j):
                if s == 0:
                    return bt0k[k][:, j * 512 : (j + 1) * 512]
                return bts[s][:, k * chunk + j * 512 : k * chunk + (j + 1) * 512]

            for m in range(mt):
                wv = winpool.tile([128, nseg * TOP], F16, tag="wval")
                wi = winpool.tile([128, nseg * TOP], U32, tag="widx")
                for s in range(nseg):
                    g = m * nseg + s
                    ps = ppool.tile([128, chunk], F32, tag="score")
                    for k in range(kt):
                        for j in range(nsub):
                            nc.tensor.matmul(
                                ps[:, j * 512 : (j + 1) * 512],
                                at_sb[:, k * na + m * 128 : k * na + (m + 1) * 128],
                                bt_slice(s, k, j),
                                start=(k == 0),
                                stop=(k == kt - 1),
                            )
                    s16 = spool.tile([128, chunk], F16, tag="s16")
                    nc.scalar.copy(out=s16, in_=ps)
                    csl = crep[:, s * chunk : (s + 1) * chunk]
                    # Alternate the bias subtract between DVE and GPSIMD to
                    # balance engine load (DVE ~1.5us, GPS ~4us per chunk).
                    if g % 2 == 0:
                        nc.vector.tensor_sub(s16, s16, csl)
                    else:
                        nc.gpsimd.tensor_sub(s16, s16, csl)
                    m2 = mpool.tile([128, half], F16, tag="m2")
                    nc.vector.tensor_max(m2, s16[:, :half], s16[:, half:])
                    m4 = mpool.tile([128, quad], F16, tag="m4")
                    nc.vector.tensor_max(m4, m2[:, :quad], m2[:, quad:])
                    nc.vector.max(out=wv[:, s * TOP : (s + 1) * TOP], in_=m4)
                    nc.vector.max_index(
                        out=wi[:, s * TOP : (s + 1) * TOP],
                        in_max=wv[:, s * TOP : (s + 1) * TOP],
                        in_values=m4,
                    )
                nc.sync.dma_start(out=out_val[m * 128 : (m + 1) * 128, :], in_=wv)
                nc.sync.dma_start(out=out_idx[m * 128 : (m + 1) * 128, :], in_=wi)
    nc.compile()
    return nc


def make_in_maps(a, b):
    import ml_dtypes

    kt = D // 128
    aT2 = (2.0 * a).T.astype(ml_dtypes.bfloat16)      # [512, NA]
    atp = np.ascontiguousarray(
        np.concatenate([aT2[k * 128 : (k + 1) * 128, :] for k in range(kt)], axis=1)
    )                                                 # [128, kt*NA]
    bT_full = b.T.astype(ml_dtypes.bfloat16)          # [512, NB]
    b2 = np.einsum("ij,ij->i", b, b)
    sb = b.sum(axis=1)
    c = (b2 - np.float32(2.0 * EPS) * sb).astype(np.float32)
    nseg = NB_SHARD // CHUNK
    in_maps = []
    for core in range(NCORES):
        sl = slice(core * NB_SHARD, (core + 1) * NB_SHARD)
        bT = bT_full[:, sl]
        im = {
            "at": atp,
            "crep": np.ascontiguousarray(
                np.broadcast_to(
                    c[sl].astype(np.float16)[None, :], (128, NB_SHARD)
                )
            ),
        }
        cols0 = bT[:, 0:CHUNK]
        for k in range(kt):
            im[f"bt0k{k}"] = np.ascontiguousarray(cols0[k * 128 : (k + 1) * 128, :])
        for g in range(1, nseg):
            cols = bT[:, g * CHUNK : (g + 1) * CHUNK]  # [512, CHUNK]
            im[f"bt{g}"] = np.ascontiguousarray(
                np.concatenate(
                    [cols[k * 128 : (k + 1) * 128, :] for k in range(kt)], axis=1
                )
            )
        in_maps.append(im)
    return in_maps


def merge_results(a, b, n, b_batch_size, results):
    """Gather per-core quad winners, expand each to its 4 possible columns,
    refine with the exact fp32 reference distance, pick final top-n
    (ties -> lowest index), apply the reference's buggy index bookkeeping."""
    nseg = NB_SHARD // CHUNK
    cand = []
    for core in range(NCORES):
        qi = results[core]["out_idx"].astype(np.int64)  # [NA, nseg*TOP] in [0,QUAD)
        for s in range(nseg):
            qi[:, s * TOP : (s + 1) * TOP] += core * NB_SHARD + s * CHUNK
        for off in (0, QUAD, 2 * QUAD, 3 * QUAD):
            cand.append(qi + off)
    cand = np.concatenate(cand, axis=1)  # [NA, 4*NCORES*nseg*TOP]

    a2 = np.sum(a * a, axis=1)
    sa = np.sum(a, axis=1)
    b2 = np.sum(b * b, axis=1)
    sb = np.sum(b, axis=1)
    na, d = a.shape
    out = np.empty((na, n), dtype=np.int64)
    CHQ = 128
    eps = np.float32(EPS)
    for q0 in range(0, na, CHQ):
        q1 = min(q0 + CHQ, na)
        Cc = cand[q0:q1]
        Bc = b[Cc]
        cross = np.einsum("qd,qkd->qk", a[q0:q1], Bc).astype(np.float32)
        sq = (
            a2[q0:q1, None]
            + b2[Cc]
            - np.float32(2.0) * cross
            + np.float32(2.0) * eps * (sa[q0:q1, None] - sb[Cc])
            + np.float32(d) * eps * eps
        )
        dist = np.sqrt(np.maximum(sq, np.float32(0.0)))
        ordr = np.lexsort((Cc, dist), axis=1)[:, :n]
        rows = np.arange(q1 - q0)[:, None]
        out[q0:q1] = Cc[rows, ordr]
    buggy = (out % b_batch_size) + (out // b_batch_size)
    return buggy.astype(np.int32)


def kernel(a, b, n, b_batch_size, trace=False):
    from concourse.bass_utils import run_bass_kernel_spmd

    a = np.ascontiguousarray(np.asarray(a, dtype=np.float32))
    b = np.ascontiguousarray(np.asarray(b, dtype=np.float32))
    n = int(n)
    b_batch_size = int(b_batch_size)

    nc = build_kernel()
    in_maps = make_in_maps(a, b)
    res = run_bass_kernel_spmd(
        nc, in_maps, core_ids=list(range(NCORES)), trace=trace
    )
    out = merge_results(a, b, n, b_batch_size, res.results)
    if trace:
        return out, res
    return out


# revision 30
# speedup vs baseline: 1.2728x; 1.2728x over previous
"""Sharded kNN (ArgDistanceMeasure) on 8 TRN2 NeuronCores.

Strategy (FAISS-style sharded kNN):
  - b (the database, [65536, 512]) is sharded row-wise across 8 cores
    (8192 rows each); a (queries, [2048, 512]) is replicated.
  - Ranking identity: argmin_j ||a_i - b_j + eps||^2 over j only needs the
    column-dependent part  score[i,j] = 2*a_i.b_j - (||b_j||^2 - 2*eps*sum(b_j)),
    maximized.  The row-constant terms (||a_i||^2 etc.) don't affect per-row
    ranking.
  - Per [128 queries x 2048 cols] chunk (four-engine pipeline, all ~235us):
      PE:  bf16 GEMM accumulating 2*cross into PSUM (4 K-tiles, N=512).
      ACT: copy PSUM -> SBUF, casting to fp16.
      DVE/GPS (alternating): subtract the replicated per-column bias (fp16).
      DVE: two pairwise-max levels (2048 -> 1024 -> 512, fp16 TT 2x mode),
           then max8 + find_index8 over the 512 quad-maxima.
  - Each winner expands to its 4 possible columns on the host, which
    recomputes the exact fp32 reference distance for the ~1024
    candidates/query, picks the final top-n with the reference's tie-break,
    and applies the reference's buggy index bookkeeping.  (bf16 GEMM noise +
    fp16 quantization + quad-expansion are provably safe on this data: zero
    true top-16 members lost in simulation.)
"""

import numpy as np

NA, D, NB = 2048, 512, 65536
NCORES = 8
NB_SHARD = NB // NCORES  # 8192
CHUNK = 2048             # chunk width (4 PSUM banks)
QUAD = CHUNK // 4        # 512 quad-maxima per chunk
TOP = 8                  # top-8 per chunk (vector.max width)
EPS = 1e-6


def build_kernel(na=NA, nb_shard=NB_SHARD, chunk=CHUNK):
    import concourse.mybir as mybir
    from concourse import bacc
    from concourse.tile import TileContext

    BF = mybir.dt.bfloat16
    F16 = mybir.dt.float16
    F32 = mybir.dt.float32
    U32 = mybir.dt.uint32

    nseg = nb_shard // chunk
    nsub = chunk // 512
    half = chunk // 2
    quad = chunk // 4
    kt = D // 128
    mt = na // 128

    # Bacc (not plain Bass): its compile() pipeline moves matmul waits onto
    # ldweights and splits multi-wait sync via event semaphores — TRN2
    # instructions encode at most ONE sync wait.
    nc = bacc.Bacc()

    # bT is packed chunk-column-major (all k-tiles of one 2048-column chunk
    # group contiguous) and split into one DRAM param + DMA per chunk group,
    # so the PE can start on chunk 0 long before the whole database loads.
    # Chunk group 0 is additionally split per k-tile for the earliest start.
    bt0_p = [
        nc.declare_dram_parameter(f"bt0k{k}", [128, chunk], BF, isOutput=False)
        for k in range(kt)
    ]
    bts_p = [
        nc.declare_dram_parameter(f"bt{g}", [128, kt * chunk], BF, isOutput=False)
        for g in range(1, nseg)
    ]
    at_p = nc.declare_dram_parameter("at", [128, kt * na], BF, isOutput=False)
    # Per-column bias replicated across partitions, fp16.
    crep_p = nc.declare_dram_parameter("crep", [128, nb_shard], F16, isOutput=False)
    out_val = nc.declare_dram_parameter("out_val", [na, nseg * TOP], F16, isOutput=True)
    out_idx = nc.declare_dram_parameter("out_idx", [na, nseg * TOP], U32, isOutput=True)

    with TileContext(nc) as tc:
        with (
            tc.tile_pool(name="weights", bufs=1) as wpool,
            tc.tile_pool(name="psum", bufs=2, space="PSUM") as ppool,
            tc.tile_pool(name="scores", bufs=6) as spool,
            tc.tile_pool(name="pairs", bufs=6) as mpool,
            tc.tile_pool(name="win", bufs=6) as winpool,
        ):
            at_sb = wpool.tile([128, kt * na], BF, tag="at")
            nc.sync.dma_start(out=at_sb, in_=at_p[:, :])
            bt0k = []
            for k in range(kt):
                t = wpool.tile([128, chunk], BF, tag=f"bt0k{k}")
                nc.sync.dma_start(out=t, in_=bt0_p[k][:, :])
                bt0k.append(t)
            crep = wpool.tile([128, nb_shard], F16, tag="crep")
            nc.sync.dma_start(out=crep, in_=crep_p[:, :])
            bts = [None]
            for g in range(1, nseg):
                t = wpool.tile([128, kt * chunk], BF, tag=f"bt{g}")
                nc.sync.dma_start(out=t, in_=bts_p[g - 1][:, :])
                bts.append(t)

            def bt_slice(s, k, j):
                if s == 0:
                    return bt0k[k][:, j * 512 : (j + 1) * 512]
                return bts[s][:, k * chunk + j * 512 : k * chunk + (j + 1) * 512]

            for m in range(mt):
                wv = winpool.tile([128, nseg * TOP], F16, tag="wval")
                wi = winpool.tile([128, nseg * TOP], U32, tag="widx")
                for s in range(nseg):
                    g = m * nseg + s
                    ps = ppool.tile([128, chunk], F32, tag="score")
                    for k in range(kt):
                        for j in range(nsub):
                            nc.tensor.matmul(
                                ps[:, j * 512 : (j + 1) * 512],
                                at_sb[:, k * na + m * 128 : k * na + (m + 1) * 128],
                                bt_slice(s, k, j),
                                start=(k == 0),
                                stop=(k == kt - 1),
                            )
                    s16 = spool.tile([128, chunk], F16, tag="s16")
                    nc.scalar.copy(out=s16, in_=ps)
                    csl = crep[:, s * chunk : (s + 1) * chunk]
                    # Bias subtract on DVE (fp16 2x mode, ~1.1us).  GPSIMD is
                    # deliberately NOT used: it shares SBUF ports with the DVE
                    # and concurrent Pool tensor ops slow DVE ops ~6x.
                    nc.vector.tensor_sub(s16, s16, csl)
                    m2 = mpool.tile([128, half], F16, tag="m2")
                    nc.vector.tensor_max(m2, s16[:, :half], s16[:, half:])
                    m4 = mpool.tile([128, quad], F16, tag="m4")
                    nc.vector.tensor_max(m4, m2[:, :quad], m2[:, quad:])
                    nc.vector.max(out=wv[:, s * TOP : (s + 1) * TOP], in_=m4)
                    nc.vector.max_index(
                        out=wi[:, s * TOP : (s + 1) * TOP],
                        in_max=wv[:, s * TOP : (s + 1) * TOP],
                        in_values=m4,
                    )
                nc.sync.dma_start(out=out_val[m * 128 : (m + 1) * 128, :], in_=wv)
                nc.sync.dma_start(out=out_idx[m * 128 : (m + 1) * 128, :], in_=wi)
    nc.compile()
    return nc


def make_in_maps(a, b):
    import ml_dtypes

    kt = D // 128
    aT2 = (2.0 * a).T.astype(ml_dtypes.bfloat16)      # [512, NA]
    atp = np.ascontiguousarray(
        np.concatenate([aT2[k * 128 : (k + 1) * 128, :] for k in range(kt)], axis=1)
    )                                                 # [128, kt*NA]
    bT_full = b.T.astype(ml_dtypes.bfloat16)          # [512, NB]
    b2 = np.einsum("ij,ij->i", b, b)
    sb = b.sum(axis=1)
    c = (b2 - np.float32(2.0 * EPS) * sb).astype(np.float32)
    nseg = NB_SHARD // CHUNK
    in_maps = []
    for core in range(NCORES):
        sl = slice(core * NB_SHARD, (core + 1) * NB_SHARD)
        bT = bT_full[:, sl]
        im = {
            "at": atp,
            "crep": np.ascontiguousarray(
                np.broadcast_to(
                    c[sl].astype(np.float16)[None, :], (128, NB_SHARD)
                )
            ),
        }
        cols0 = bT[:, 0:CHUNK]
        for k in range(kt):
            im[f"bt0k{k}"] = np.ascontiguousarray(cols0[k * 128 : (k + 1) * 128, :])
        for g in range(1, nseg):
            cols = bT[:, g * CHUNK : (g + 1) * CHUNK]  # [512, CHUNK]
            im[f"bt{g}"] = np.ascontiguousarray(
                np.concatenate(
                    [cols[k * 128 : (k + 1) * 128, :] for k in range(kt)], axis=1
                )
            )
        in_maps.append(im)
    return in_maps


def merge_results(a, b, n, b_batch_size, results):
    """Gather per-core quad winners, expand each to its 4 possible columns,
    refine with the exact fp32 reference distance, pick final top-n
    (ties -> lowest index), apply the reference's buggy index bookkeeping."""
    nseg = NB_SHARD // CHUNK
    cand = []
    for core in range(NCORES):
        qi = results[core]["out_idx"].astype(np.int64)  # [NA, nseg*TOP] in [0,QUAD)
        for s in range(nseg):
            qi[:, s * TOP : (s + 1) * TOP] += core * NB_SHARD + s * CHUNK
        for off in (0, QUAD, 2 * QUAD, 3 * QUAD):
            cand.append(qi + off)
    cand = np.concatenate(cand, axis=1)  # [NA, 4*NCORES*nseg*TOP]

    a2 = np.sum(a * a, axis=1)
    sa = np.sum(a, axis=1)
    b2 = np.sum(b * b, axis=1)
    sb = np.sum(b, axis=1)
    na, d = a.shape
    out = np.empty((na, n), dtype=np.int64)
    CHQ = 128
    eps = np.float32(EPS)
    for q0 in range(0, na, CHQ):
        q1 = min(q0 + CHQ, na)
        Cc = cand[q0:q1]
        Bc = b[Cc]
        cross = np.einsum("qd,qkd->qk", a[q0:q1], Bc).astype(np.float32)
        sq = (
            a2[q0:q1, None]
            + b2[Cc]
            - np.float32(2.0) * cross
            + np.float32(2.0) * eps * (sa[q0:q1, None] - sb[Cc])
            + np.float32(d) * eps * eps
        )
        dist = np.sqrt(np.maximum(sq, np.float32(0.0)))
        ordr = np.lexsort((Cc, dist), axis=1)[:, :n]
        rows = np.arange(q1 - q0)[:, None]
        out[q0:q1] = Cc[rows, ordr]
    buggy = (out % b_batch_size) + (out // b_batch_size)
    return buggy.astype(np.int32)


def kernel(a, b, n, b_batch_size, trace=False):
    from concourse.bass_utils import run_bass_kernel_spmd

    a = np.ascontiguousarray(np.asarray(a, dtype=np.float32))
    b = np.ascontiguousarray(np.asarray(b, dtype=np.float32))
    n = int(n)
    b_batch_size = int(b_batch_size)

    nc = build_kernel()
    in_maps = make_in_maps(a, b)
    res = run_bass_kernel_spmd(
        nc, in_maps, core_ids=list(range(NCORES)), trace=trace
    )
    out = merge_results(a, b, n, b_batch_size, res.results)
    if trace:
        return out, res
    return out


# revision 35
# speedup vs baseline: 1.3655x; 1.0729x over previous
"""Sharded kNN (ArgDistanceMeasure) on 8 TRN2 NeuronCores.

Strategy (FAISS-style sharded kNN):
  - b (the database, [65536, 512]) is sharded row-wise across 8 cores
    (8192 rows each); a (queries, [2048, 512]) is replicated.
  - Ranking identity: argmin_j ||a_i - b_j + eps||^2 over j only needs the
    column-dependent part  score[i,j] = 2*a_i.b_j - (||b_j||^2 - 2*eps*sum(b_j)),
    maximized.  The row-constant terms (||a_i||^2 etc.) don't affect per-row
    ranking.
  - Per [128 queries x 2048 cols] chunk (four-engine pipeline, all ~235us):
      PE:  bf16 GEMM accumulating 2*cross into PSUM (4 K-tiles, N=512).
      ACT: copy PSUM -> SBUF, casting to fp16.
      DVE/GPS (alternating): subtract the replicated per-column bias (fp16).
      DVE: two pairwise-max levels (2048 -> 1024 -> 512, fp16 TT 2x mode),
           then max8 + find_index8 over the 512 quad-maxima.
  - Each winner expands to its 4 possible columns on the host, which
    recomputes the exact fp32 reference distance for the ~1024
    candidates/query, picks the final top-n with the reference's tie-break,
    and applies the reference's buggy index bookkeeping.  (bf16 GEMM noise +
    fp16 quantization + quad-expansion are provably safe on this data: zero
    true top-16 members lost in simulation.)
"""

import numpy as np

NA, D, NB = 2048, 512, 65536
NCORES = 8
NB_SHARD = NB // NCORES  # 8192
CHUNK = 2048             # chunk width (4 PSUM banks)
QUAD = CHUNK // 4        # 512 quad-maxima per chunk
TOP = 8                  # top-8 per chunk (vector.max width)
EPS = 1e-6


def build_kernel(na=NA, nb_shard=NB_SHARD, chunk=CHUNK):
    import concourse.mybir as mybir
    from concourse import bacc
    from concourse.tile import TileContext

    BF = mybir.dt.bfloat16
    F16 = mybir.dt.float16
    F32 = mybir.dt.float32
    U32 = mybir.dt.uint32

    nseg = nb_shard // chunk
    nsub = chunk // 512
    half = chunk // 2
    quad = chunk // 4
    kt = D // 128
    mt = na // 128

    # Bacc (not plain Bass): its compile() pipeline moves matmul waits onto
    # ldweights and splits multi-wait sync via event semaphores — TRN2
    # instructions encode at most ONE sync wait.
    nc = bacc.Bacc()

    # bT is packed chunk-column-major (all k-tiles of one 2048-column chunk
    # group contiguous) and split into one DRAM param + DMA per chunk group,
    # so the PE can start on chunk 0 long before the whole database loads.
    # Chunk group 0 is additionally split per k-tile for the earliest start.
    bt0_p = [
        nc.declare_dram_parameter(f"bt0k{k}", [128, chunk], BF, isOutput=False)
        for k in range(kt)
    ]
    bts_p = [
        nc.declare_dram_parameter(f"bt{g}", [128, kt * chunk], BF, isOutput=False)
        for g in range(1, nseg)
    ]
    at_kp = [
        nc.declare_dram_parameter(f"atk{k}", [128, na], BF, isOutput=False)
        for k in range(kt)
    ]
    # Per-column bias replicated across partitions, fp16.
    crep_p = nc.declare_dram_parameter("crep", [128, nb_shard], F16, isOutput=False)
    out_val = nc.declare_dram_parameter("out_val", [na, nseg * TOP], F16, isOutput=True)
    out_idx = nc.declare_dram_parameter("out_idx", [na, nseg * TOP], U32, isOutput=True)

    with TileContext(nc) as tc:
        with (
            tc.tile_pool(name="weights", bufs=1) as wpool,
            tc.tile_pool(name="psum", bufs=2, space="PSUM") as ppool,
            tc.tile_pool(name="scores", bufs=6) as spool,
            tc.tile_pool(name="pairs", bufs=6) as mpool,
            tc.tile_pool(name="win", bufs=6) as winpool,
        ):
            # Interleave at/bt0 k-tile loads so the first chunk's k=0 matmuls
            # can start after ~1MB has landed.
            at_k = []
            bt0k = []
            for k in range(kt):
                ta = wpool.tile([128, na], BF, tag=f"atk{k}")
                nc.sync.dma_start(out=ta, in_=at_kp[k][:, :])
                at_k.append(ta)
                tb = wpool.tile([128, chunk], BF, tag=f"bt0k{k}")
                nc.sync.dma_start(out=tb, in_=bt0_p[k][:, :])
                bt0k.append(tb)
            crep = wpool.tile([128, nb_shard], F16, tag="crep")
            nc.sync.dma_start(out=crep, in_=crep_p[:, :])
            bts = [None]
            for g in range(1, nseg):
                t = wpool.tile([128, kt * chunk], BF, tag=f"bt{g}")
                nc.sync.dma_start(out=t, in_=bts_p[g - 1][:, :])
                bts.append(t)

            def bt_slice(s, k, j):
                if s == 0:
                    return bt0k[k][:, j * 512 : (j + 1) * 512]
                return bts[s][:, k * chunk + j * 512 : k * chunk + (j + 1) * 512]

            # Winner tiles for all 16 m-tiles stay alive across the whole
            # kernel (3KB/partition total); the s-outer loop order means the
            # first 16 chunks only need bt0+at, so the DMA of bt1..3 hides
            # under ~55us of PE work.
            wvs = [
                winpool.tile([128, nseg * TOP], F16, tag=f"wval{m}", name=f"wval{m}")
                for m in range(mt)
            ]
            wis = [
                winpool.tile([128, nseg * TOP], U32, tag=f"widx{m}", name=f"widx{m}")
                for m in range(mt)
            ]

            for s in range(nseg):
                csl = crep[:, s * chunk : (s + 1) * chunk]
                for m in range(mt):
                    ps = ppool.tile([128, chunk], F32, tag="score")
                    for k in range(kt):
                        for j in range(nsub):
                            nc.tensor.matmul(
                                ps[:, j * 512 : (j + 1) * 512],
                                at_k[k][:, m * 128 : (m + 1) * 128],
                                bt_slice(s, k, j),
                                start=(k == 0),
                                stop=(k == kt - 1),
                            )
                    s16 = spool.tile([128, chunk], F16, tag="s16")
                    nc.scalar.copy(out=s16, in_=ps)
                    # Bias subtract on DVE (fp16 2x mode, ~1.1us).  GPSIMD is
                    # deliberately NOT used: it shares SBUF ports with the DVE
                    # and concurrent Pool tensor ops slow DVE ops ~6x.
                    nc.vector.tensor_sub(s16, s16, csl)
                    m2 = mpool.tile([128, half], F16, tag="m2")
                    nc.vector.tensor_max(m2, s16[:, :half], s16[:, half:])
                    m4 = mpool.tile([128, quad], F16, tag="m4")
                    nc.vector.tensor_max(m4, m2[:, :quad], m2[:, quad:])
                    nc.vector.max(out=wvs[m][:, s * TOP : (s + 1) * TOP], in_=m4)
                    nc.vector.max_index(
                        out=wis[m][:, s * TOP : (s + 1) * TOP],
                        in_max=wvs[m][:, s * TOP : (s + 1) * TOP],
                        in_values=m4,
                    )
            for m in range(mt):
                nc.sync.dma_start(out=out_val[m * 128 : (m + 1) * 128, :], in_=wvs[m])
                nc.sync.dma_start(out=out_idx[m * 128 : (m + 1) * 128, :], in_=wis[m])
    nc.compile()
    return nc


def make_in_maps(a, b):
    import ml_dtypes

    kt = D // 128
    aT2 = (2.0 * a).T.astype(ml_dtypes.bfloat16)      # [512, NA]
    bT_full = b.T.astype(ml_dtypes.bfloat16)          # [512, NB]
    b2 = np.einsum("ij,ij->i", b, b)
    sb = b.sum(axis=1)
    c = (b2 - np.float32(2.0 * EPS) * sb).astype(np.float32)
    nseg = NB_SHARD // CHUNK
    in_maps = []
    for core in range(NCORES):
        sl = slice(core * NB_SHARD, (core + 1) * NB_SHARD)
        bT = bT_full[:, sl]
        im = {
            "crep": np.ascontiguousarray(
                np.broadcast_to(
                    c[sl].astype(np.float16)[None, :], (128, NB_SHARD)
                )
            ),
        }
        for k in range(kt):
            im[f"atk{k}"] = np.ascontiguousarray(aT2[k * 128 : (k + 1) * 128, :])
        cols0 = bT[:, 0:CHUNK]
        for k in range(kt):
            im[f"bt0k{k}"] = np.ascontiguousarray(cols0[k * 128 : (k + 1) * 128, :])
        for g in range(1, nseg):
            cols = bT[:, g * CHUNK : (g + 1) * CHUNK]  # [512, CHUNK]
            im[f"bt{g}"] = np.ascontiguousarray(
                np.concatenate(
                    [cols[k * 128 : (k + 1) * 128, :] for k in range(kt)], axis=1
                )
            )
        in_maps.append(im)
    return in_maps


def merge_results(a, b, n, b_batch_size, results):
    """Gather per-core quad winners, expand each to its 4 possible columns,
    refine with the exact fp32 reference distance, pick final top-n
    (ties -> lowest index), apply the reference's buggy index bookkeeping."""
    nseg = NB_SHARD // CHUNK
    cand = []
    for core in range(NCORES):
        qi = results[core]["out_idx"].astype(np.int64)  # [NA, nseg*TOP] in [0,QUAD)
        for s in range(nseg):
            qi[:, s * TOP : (s + 1) * TOP] += core * NB_SHARD + s * CHUNK
        for off in (0, QUAD, 2 * QUAD, 3 * QUAD):
            cand.append(qi + off)
    cand = np.concatenate(cand, axis=1)  # [NA, 4*NCORES*nseg*TOP]

    a2 = np.sum(a * a, axis=1)
    sa = np.sum(a, axis=1)
    b2 = np.sum(b * b, axis=1)
    sb = np.sum(b, axis=1)
    na, d = a.shape
    out = np.empty((na, n), dtype=np.int64)
    CHQ = 128
    eps = np.float32(EPS)
    for q0 in range(0, na, CHQ):
        q1 = min(q0 + CHQ, na)
        Cc = cand[q0:q1]
        Bc = b[Cc]
        cross = np.einsum("qd,qkd->qk", a[q0:q1], Bc).astype(np.float32)
        sq = (
            a2[q0:q1, None]
            + b2[Cc]
            - np.float32(2.0) * cross
            + np.float32(2.0) * eps * (sa[q0:q1, None] - sb[Cc])
            + np.float32(d) * eps * eps
        )
        dist = np.sqrt(np.maximum(sq, np.float32(0.0)))
        ordr = np.lexsort((Cc, dist), axis=1)[:, :n]
        rows = np.arange(q1 - q0)[:, None]
        out[q0:q1] = Cc[rows, ordr]
    buggy = (out % b_batch_size) + (out // b_batch_size)
    return buggy.astype(np.int32)


def kernel(a, b, n, b_batch_size, trace=False):
    from concourse.bass_utils import run_bass_kernel_spmd

    a = np.ascontiguousarray(np.asarray(a, dtype=np.float32))
    b = np.ascontiguousarray(np.asarray(b, dtype=np.float32))
    n = int(n)
    b_batch_size = int(b_batch_size)

    nc = build_kernel()
    in_maps = make_in_maps(a, b)
    res = run_bass_kernel_spmd(
        nc, in_maps, core_ids=list(range(NCORES)), trace=trace
    )
    out = merge_results(a, b, n, b_batch_size, res.results)
    if trace:
        return out, res
    return out
